# revision 11
# baseline (speedup 1.0000x reference)
"""Fused multi-head tanh-attention kernel for Trainium2 (8 NeuronCores).

Problem: y[s,b,:] = concat_h( softmax_t(tanh(q_h k_h^T / 8) - 10000*(1-mask)) @ v_h )
with q/k/v = per-head projections of x.  Shapes: x [1024,16,512], mask [16,1024],
w* [8,64,512] -> y [1024,16,512].

Strategy: batch-parallel over 8 cores (2 batches per core).  Per core, a fully
fused flash-style pipeline keeps the [S,S] score matrices in PSUM/SBUF only:
  - x and the weights are transposed on-chip via PE-transpose (contraction dim
    on partitions); all matmuls run in float32r (full PE rate, ~1e-4 rel err),
  - scores are built in scoresT [t,s] layout; the key mask is folded into the
    v tiles (v rows and the appended ones-columns are scaled by mask), so
    masked keys contribute exactly 0 to both numerator and denominator --
    matching the reference's exp(-10000) == 0 underflow exactly,
  - tanh+exp run on the scalar engine (one table set holds both; exp is
    emitted once per two t-chunks to amortize instruction overhead), PV
    accumulates unnormalized out^T plus the softmax denominator (ones-column
    trick), which is PE-transposed back and divided on the vector engine,
  - the scalar engine is the bottleneck (~250us busy), so all other work
    (projections, v construction, next batch's x transposes, the previous
    head's epilogue, weight transposes) is emitted interleaved into the
    attention t-loop as background tasks so the in-order engine queues never
    starve the activation engine.
"""

import sys

sys.path.insert(0, "/opt/trn_rl_repo")

from contextlib import ExitStack

import numpy as np

S, B, D, H, DH = 1024, 16, 512, 8, 64
NCORES = 8
BPC = B // NCORES  # batches per core
SC = S // 128  # 8 s-chunks (and t-chunks)
DC = D // 128  # 4 d-chunks

_compiled_nc = None


def _make_pools(tc, ctx):
    pools = {}
    pools["singles"] = ctx.enter_context(tc.tile_pool(name="singles", bufs=1))
    pools["nat"] = ctx.enter_context(tc.tile_pool(name="nat", bufs=3))
    pools["qk"] = ctx.enter_context(tc.tile_pool(name="qk", bufs=4))
    pools["vh"] = ctx.enter_context(tc.tile_pool(name="vh", bufs=8))
    pools["tanh"] = ctx.enter_context(tc.tile_pool(name="tanh", bufs=3))
    pools["exp"] = ctx.enter_context(tc.tile_pool(name="exp", bufs=3))
    pools["outT"] = ctx.enter_context(tc.tile_pool(name="outT", bufs=2))
    pools["outsb"] = ctx.enter_context(tc.tile_pool(name="outsb", bufs=2))
    pools["small"] = ctx.enter_context(tc.tile_pool(name="small", bufs=4))
    # PSUM: 8 banks.  ps_big ([128,1024]f32 slots = 2 banks, bufs=3 = 6
    # banks) rotates scores / projections / v chunks / all transposes.
    # ps_o (2 banks, bufs=1) holds the per-head PV accumulator.
    pools["ps_big"] = ctx.enter_context(
        tc.tile_pool(name="ps_big", bufs=3, space="PSUM")
    )
    pools["ps_o"] = ctx.enter_context(tc.tile_pool(name="ps_o", bufs=1, space="PSUM"))
    return pools


def _emit(nc, tc, pools, tile, mybir, aps, u=0):
    f32 = mybir.dt.float32
    f32r = mybir.dt.float32r
    AF = mybir.ActivationFunctionType
    Alu = mybir.AluOpType
    x_d, mask_d, wq_d, wk_d, wv_d, id_d, y_d = aps

    singles = pools["singles"]
    nat = pools["nat"]
    qk_pool = pools["qk"]
    vh_pool = pools["vh"]
    tanh_pool = pools["tanh"]
    exp_pool = pools["exp"]
    outT_pool = pools["outT"]
    outsb_pool = pools["outsb"]
    small = pools["small"]
    ps_big = pools["ps_big"]
    ps_o = pools["ps_o"]

    yr = y_d.rearrange("(c p) b e -> p c b e", p=128)

    # ---------------- prologue ------------------------------------------
    ident = singles.tile([128, 128], f32r, tag="ident", name=f"ident_u{u}")
    nc.sync.dma_start(ident, id_d)
    fill64 = singles.tile([128, SC, 64], f32, tag="fill64", name=f"fill64_u{u}")
    nc.vector.memset(fill64, 1.0)

    # mask columns [128, SC] per batch (t = tck*128 + partition) and the
    # mask broadcast over 64 cols (fills vh's ones-block)
    msk = {}
    for b in range(BPC):
        m = small.tile([128, SC], f32, tag="msk", name=f"msk{b}_u{u}")
        nc.sync.dma_start(m, mask_d[b].rearrange("(c p) -> p c", p=128))
        msk[b] = m
        mf = singles.tile([128, SC, 64], f32, tag=f"mfill{b}", name=f"mfill{b}_u{u}")
        for tck in range(SC):
            nc.vector.tensor_scalar(
                mf[:, tck, :], fill64[:, tck, :], m[:, tck : tck + 1], None, Alu.mult
            )
        msk[b, "fill"] = mf

    # ---- emitters ------------------------------------------------------
    wTq = {}
    wTk = {}
    wTv = {}

    def emit_wqk_tr(nm, w_d, wT, hp):
        w_nat = nat.tile([128, D], f32r, tag="nat", name=f"w_nat_u{u}")
        nc.sync.dma_start(w_nat, w_d[2 * hp : 2 * hp + 2].rearrange("h e d -> (h e) d"))
        wt = singles.tile(
            [128, DC, 128], f32r, tag=f"wT{nm}{hp}", name=f"wT{nm}{hp}_u{u}"
        )
        wT[hp] = wt
        for dc in range(DC):
            pst = ps_big.tile([128, 128], f32r, tag="ps_big", name=f"pstr_u{u}")
            nc.tensor.transpose(pst, w_nat[:, dc * 128 : dc * 128 + 128], ident)
            nc.vector.tensor_copy(wt[:, dc, :], pst)

    def emit_wv_tr(q4):
        wt = singles.tile([128, DC, 256], f32r, tag=f"wTv{q4}", name=f"wTv{q4}_u{u}")
        wTv[q4] = wt
        for half in range(2):
            w_nat = nat.tile([128, D], f32r, tag="nat", name=f"w_nat_u{u}")
            h0 = 4 * q4 + 2 * half
            nc.sync.dma_start(w_nat, wv_d[h0 : h0 + 2].rearrange("h e d -> (h e) d"))
            for dc in range(DC):
                pst = ps_big.tile([128, 128], f32r, tag="ps_big", name=f"pstr_u{u}")
                nc.tensor.transpose(pst, w_nat[:, dc * 128 : dc * 128 + 128], ident)
                nc.vector.tensor_copy(wt[:, dc, half * 128 : half * 128 + 128], pst)

    xbT = {}

    def alloc_xbT(b):
        for dc in range(DC):
            xbT[b, dc] = singles.tile(
                [128, S], f32r, tag=f"xbT{b}{dc}", name=f"xbT{b}{dc}_u{u}"
            )

    def emit_x_tr(b, sc):
        x_nat = nat.tile([128, D], f32r, tag="nat", name=f"x_nat_u{u}")
        nc.sync.dma_start(x_nat, x_d[sc * 128 : sc * 128 + 128, b, :])
        for dc in range(DC):
            pst = ps_big.tile([128, 128], f32r, tag="ps_big", name=f"pstr_u{u}")
            nc.tensor.transpose(pst, x_nat[:, dc * 128 : dc * 128 + 128], ident)
            nc.vector.tensor_copy(xbT[b, dc][:, sc * 128 : sc * 128 + 128], pst)

    qkT = {}

    def emit_proj_qk(b, hp, nm):
        wT = wTq if nm == "q" else wTk
        psp = ps_big.tile([128, S], f32, tag="ps_big", name=f"psp_u{u}")
        for dc in range(DC):
            for sh in range(2):
                nc.tensor.matmul(
                    psp[:, sh * 512 : sh * 512 + 512],
                    wT[hp][:, dc, :],
                    xbT[b, dc][:, sh * 512 : sh * 512 + 512],
                    start=(dc == 0),
                    stop=(dc == DC - 1),
                )
        t = qk_pool.tile([128, S], f32r, tag="qkT", name=f"qkT{nm}_u{u}")
        qkT[b, hp, nm] = t
        nc.vector.tensor_copy(t, psp)

    vh = {}

    def alloc_vh(b, q4):
        for h in range(4 * q4, 4 * q4 + 4):
            vh[b, h] = vh_pool.tile(
                [128, SC, 128], f32r, tag="vh", name=f"vh{b}_{h}_u{u}"
            )
            nc.vector.tensor_copy(vh[b, h][:, :, 64:128], msk[b, "fill"])

    def emit_v_chunk(b, q4, tck):
        psv = ps_big.tile([128, 256], f32, tag="ps_big", name=f"psv_u{u}")
        for dc in range(DC):
            nc.tensor.matmul(
                psv,
                xbT[b, dc][:, tck * 128 : tck * 128 + 128],
                wTv[q4][:, dc, :],
                start=(dc == 0),
                stop=(dc == DC - 1),
            )
        for h_in, h in enumerate(range(4 * q4, 4 * q4 + 4)):
            nc.vector.tensor_scalar(
                vh[b, h][:, tck, 0:64],
                psv[:, h_in * 64 : h_in * 64 + 64],
                msk[b][:, tck : tck + 1],
                None,
                Alu.mult,
            )

    def out_stage_parts(b, h, pso):
        state = {}

        def p1():
            outT = outT_pool.tile([128, S], f32r, tag="outT", name=f"outT_u{u}")
            nc.vector.tensor_copy(outT, pso)
            state["outT"] = outT

        def p2():
            pst = ps_big.tile([128, SC, 128], f32r, tag="ps_big", name=f"psto_u{u}")
            for sc in range(SC):
                nc.tensor.transpose(
                    pst[:, sc, 0:128], state["outT"][:, sc * 128 : sc * 128 + 128], ident
                )
            state["pst"] = pst

        def p3():
            pst = state["pst"]
            rec = small.tile([128, SC], f32, tag="rec", name=f"rec_u{u}")
            nc.vector.reciprocal(rec, pst[:, :, 64])
            osb = outsb_pool.tile([128, SC, 64], f32, tag="osb", name=f"osb_u{u}")
            for sc in range(SC):
                nc.vector.tensor_scalar(
                    osb[:, sc, :], pst[:, sc, 0:64], rec[:, sc : sc + 1], None, Alu.mult
                )
            nc.sync.dma_start(yr[:, :, b, h * 64 : h * 64 + 64], osb)

        return [p1, p2, p3]

    # ---------------- bootstrap -----------------------------------------
    emit_wqk_tr("q", wq_d, wTq, 0)
    emit_wqk_tr("k", wk_d, wTk, 0)
    emit_wv_tr(0)
    alloc_xbT(0)
    for sc in range(SC):
        emit_x_tr(0, sc)
    emit_proj_qk(0, 0, "q")
    emit_proj_qk(0, 0, "k")
    alloc_vh(0, 0)
    for tck in range(4):
        emit_v_chunk(0, 0, tck)

    # background task lists per head index
    NH = BPC * H
    bg = {i: [] for i in range(NH + 1)}
    bg[0] += [(lambda tck=tck: emit_v_chunk(0, 0, tck)) for tck in range(4, SC)]
    bg[0] += [
        (lambda hp=hp: emit_wqk_tr("q", wq_d, wTq, hp)) for hp in range(1, H // 2)
    ]
    bg[1] += [
        (lambda hp=hp: emit_wqk_tr("k", wk_d, wTk, hp)) for hp in range(1, H // 2)
    ]
    bg[1] += [lambda: emit_wv_tr(1)]
    for b in range(BPC):
        base = b * H
        for h in range(1, H, 2):
            if h < H - 1:
                hp = (h + 1) // 2
                bg[base + h] += [
                    lambda b=b, hp=hp: emit_proj_qk(b, hp, "q"),
                    lambda b=b, hp=hp: emit_proj_qk(b, hp, "k"),
                ]
        bg[base + 2] += [lambda b=b: alloc_vh(b, 1)]
        bg[base + 2] += [
            (lambda b=b, tck=tck: emit_v_chunk(b, 1, tck)) for tck in range(0, 4)
        ]
        bg[base + 3] += [
            (lambda b=b, tck=tck: emit_v_chunk(b, 1, tck)) for tck in range(4, SC)
        ]
    if BPC > 1:
        bg[4] += [lambda: alloc_xbT(1)]
        bg[4] += [(lambda sc=sc: emit_x_tr(1, sc)) for sc in range(0, 3)]
        bg[5] += [(lambda sc=sc: emit_x_tr(1, sc)) for sc in range(3, 6)]
        bg[6] += [(lambda sc=sc: emit_x_tr(1, sc)) for sc in range(6, SC)]
        bg[7] += [
            lambda: emit_proj_qk(1, 0, "q"),
            lambda: emit_proj_qk(1, 0, "k"),
            lambda: alloc_vh(1, 0),
        ]
        bg[7] += [(lambda tck=tck: emit_v_chunk(1, 0, tck)) for tck in range(SC)]

    # ---------------- main attention loop --------------------------------
    heads = [(b, h) for b in range(BPC) for h in range(H)]

    def emit_qk_chunk(b, hp, h2, tck):
        r0 = h2 * 64
        kT = qkT[b, hp, "k"]
        qT = qkT[b, hp, "q"]
        pss = ps_big.tile([128, S], f32, tag="ps_big", name=f"pss_u{u}")
        for sh in range(2):
            nc.tensor.matmul(
                pss[:, sh * 512 : sh * 512 + 512],
                kT[r0 : r0 + 64, tck * 128 : tck * 128 + 128],
                qT[r0 : r0 + 64, sh * 512 : sh * 512 + 512],
                start=True,
                stop=True,
            )
        return pss

    pending_qk = None
    for hi, (b, h) in enumerate(heads):
        hp, h2 = h // 2, h % 2
        pso = ps_o.tile([128, S], f32, tag="ps_o", name=f"pso_u{u}")
        tasks = list(bg[hi])
        done = 0
        tnh = None
        for tck in range(SC):
            if tck == 0 and pending_qk is not None:
                pss = pending_qk
                pending_qk = None
            else:
                pss = emit_qk_chunk(b, hp, h2, tck)
            if tck % 2 == 0:
                tnh = tanh_pool.tile([128, 2, S], f32, tag="tanh", name=f"tnh_u{u}")
            nc.scalar.activation(tnh[:, tck % 2, :], pss, AF.Tanh, scale=0.125)
            ex = None
            if tck % 2 == 1:
                ex = exp_pool.tile([128, 2, S], f32r, tag="exp", name=f"ex_u{u}")
                nc.scalar.activation(ex, tnh.rearrange("p a s -> p (a s)"), AF.Exp)
            # drain background work (keeps PE/DVE busy while ACT runs)
            target = (len(tasks) * (tck + 1) + SC - 1) // SC
            while done < target:
                tasks[done]()
                done += 1
            if tck == SC - 1 and hi + 1 < len(heads):
                nb, nh = heads[hi + 1]
                pending_qk = emit_qk_chunk(nb, nh // 2, nh % 2, 0)
            if tck % 2 == 1:
                for half in range(2):
                    for sh in range(2):
                        nc.tensor.matmul(
                            pso[:, sh * 512 : sh * 512 + 512],
                            vh[b, h][:, tck - 1 + half, :],
                            ex[:, half, sh * 512 : sh * 512 + 512],
                            start=(tck == 1 and half == 0),
                            stop=(tck == SC - 1 and half == 1),
                        )
        bg[hi + 1] = out_stage_parts(b, h, pso) + bg[hi + 1]
    for t in bg[NH]:
        t()


def _build(unroll=1):
    import concourse.bass as bass  # noqa: F401
    import concourse.tile as tile
    from concourse import bacc, mybir

    f32 = mybir.dt.float32
    f32r = mybir.dt.float32r
    nc = bacc.Bacc("TRN2", target_bir_lowering=False, debug=False)
    x_d = nc.dram_tensor("x", [S, BPC, D], f32r, kind="ExternalInput").ap()
    mask_d = nc.dram_tensor("mask", [BPC, S], f32, kind="ExternalInput").ap()
    wq_d = nc.dram_tensor("wq", [H, DH, D], f32r, kind="ExternalInput").ap()
    wk_d = nc.dram_tensor("wk", [H, DH, D], f32r, kind="ExternalInput").ap()
    wv_d = nc.dram_tensor("wv", [H, DH, D], f32r, kind="ExternalInput").ap()
    id_d = nc.dram_tensor("ident", [128, 128], f32r, kind="ExternalInput").ap()
    y_d = nc.dram_tensor("y", [S, BPC, D], f32, kind="ExternalOutput").ap()
    with tile.TileContext(nc) as tc, ExitStack() as ctx:
        pools = _make_pools(tc, ctx)
        for u in range(unroll):
            _emit(
                nc, tc, pools, tile, mybir, (x_d, mask_d, wq_d, wk_d, wv_d, id_d, y_d), u
            )
    nc.compile()
    return nc


def get_compiled():
    global _compiled_nc
    if _compiled_nc is None:
        _compiled_nc = _build()
    return _compiled_nc


def make_in_maps(x, mask, wq, wk, wv):
    x = np.asarray(x, np.float32)
    mask = np.asarray(mask, np.float32)
    wq = np.ascontiguousarray(np.asarray(wq, np.float32))
    wk = np.ascontiguousarray(np.asarray(wk, np.float32))
    wv = np.ascontiguousarray(np.asarray(wv, np.float32))
    ident = np.eye(128, dtype=np.float32)
    maps = []
    for c in range(NCORES):
        maps.append(
            {
                "x": np.ascontiguousarray(x[:, c * BPC : (c + 1) * BPC, :]),
                "mask": np.ascontiguousarray(mask[c * BPC : (c + 1) * BPC, :]),
                "wq": wq,
                "wk": wk,
                "wv": wv,
                "ident": ident,
            }
        )
    return maps


def kernel(x, mask, wq, wk, wv):
    from concourse.bass_utils import run_bass_kernel_spmd

    nc = get_compiled()
    in_maps = make_in_maps(x, mask, wq, wk, wv)
    res = run_bass_kernel_spmd(nc, in_maps, list(range(NCORES))).results
    y = np.concatenate([r["y"] for r in res], axis=1)
    return np.ascontiguousarray(y.astype(np.float32, copy=False))
